# revision 13
# baseline (speedup 1.0000x reference)
"""Binary-weight 3x3 conv2d (stride 1, pad 1) on 8 TRN2 NeuronCores.

Reference computes y = conv2d(x, sign(weights)) in NCHW/OIHW, f32.
  x: (32, 128, 56, 56) f32, weights: (256, 128, 3, 3) f32 -> y: (32, 256, 56, 56) f32

Strategy (sharding hint: data-parallel on batch, weights replicated):
  - 8 cores x 4 images each.
  - Host side: binarize weights (sign -> exactly representable +-1 in bf16)
    and lay out as [kh*kw, ci, co] bf16.
  - Per core: each image is zero-padded to 58x58 in SBUF (bf16); the 3x3 conv
    becomes 9 shifted matmuls (lhsT = w[k9] in [ci=128, co=128] chunks,
    rhs = padded x slice [ci=128, 8 rows x 56 cols]) accumulating into PSUM.
  - PSUM f32 tile [128, 448] -> SBUF -> DMA to y.

x is cast to bf16 on-chip (weights are exact +-1, so the only quantization is
on x: ~1.7e-3 relative error on the output; accumulation is f32 in PSUM).
"""

import numpy as np
import ml_dtypes

import concourse.bass as bass
import concourse.bacc as bacc
import concourse.mybir as mybir
import concourse.tile as tile
from concourse.bass_utils import run_bass_kernel_spmd

N_CORES = 8
B, CI, H, W = 32, 128, 56, 56
CO = 256
KH = KW = 3
BPC = B // N_CORES          # images per core
HP, WP = H + 2, W + 2       # padded spatial
ROWS_PER_TILE = 8
N_ROW_TILES = H // ROWS_PER_TILE   # 7
NFREE = ROWS_PER_TILE * W          # 448 <= 512 (one PSUM bank)

F32 = mybir.dt.float32
BF16 = mybir.dt.bfloat16


DEFAULT_CFG = dict(
    psum_bufs=6, out_bufs=6, xstage_bufs=3, xpad_bufs=3,
    copy_engine="vector",        # "vector" | "scalar" | "alternate"
    chunked=False,               # per-row-tile x chunks (finer pipeline)
)

CH_ROWS = ROWS_PER_TILE + 2     # 10 padded rows per chunk


def _alloc_chunk_tiles(tc, nc):
    """Persistent x chunk tiles with one-time zeroed pad borders.

    A chunk holds the 10 padded input rows needed by one 8-row output tile.
    kind 'first' has the top pad row, 'last' the bottom pad row. Pad columns
    (0 and 57) are zeroed once; DMAs only ever write interior cells, so the
    zeros persist across reuse.
    """
    tiles = {"first": [], "mid": [], "last": []}
    pool = tc.alloc_tile_pool(name="chunks", bufs=1)
    seq = [0]

    def mk(kind):
        i = seq[0]
        seq[0] += 1
        xs = pool.tile([CI, CH_ROWS, WP], F32, name=f"xs_{kind}{i}")
        xq = pool.tile([CI, CH_ROWS, WP], BF16, name=f"xq_{kind}{i}")
        nc.vector.memset(xs[:, :, 0], 0.0)
        nc.vector.memset(xs[:, :, WP - 1], 0.0)
        if kind == "first":
            nc.vector.memset(xs[:, 0, :], 0.0)
        if kind == "last":
            nc.vector.memset(xs[:, CH_ROWS - 1, :], 0.0)
        return (xs, xq)

    for _ in range(2):
        tiles["first"].append(mk("first"))
    for _ in range(2):
        tiles["last"].append(mk("last"))
    for _ in range(4):
        tiles["mid"].append(mk("mid"))
    return tiles, pool


def _emit_body_chunked(nc, pools, x_d, y_d, w_sb, cfg, chunk_tiles, state):
    _, _, out_pool, psum_pool = pools
    for n in range(BPC):
        for t in range(N_ROW_TILES):
            r0 = t * ROWS_PER_TILE
            kind = "first" if t == 0 else ("last" if t == N_ROW_TILES - 1 else "mid")
            group = chunk_tiles[kind]
            xs, xq = group[state[kind] % len(group)]
            state[kind] += 1
            if kind == "first":
                nc.sync.dma_start(xs[:, 1:CH_ROWS, 1:W + 1],
                                  x_d[n, :, 0:CH_ROWS - 1, :])
            elif kind == "last":
                nc.sync.dma_start(xs[:, 0:CH_ROWS - 1, 1:W + 1],
                                  x_d[n, :, r0 - 1:H, :])
            else:
                nc.sync.dma_start(xs[:, :, 1:W + 1],
                                  x_d[n, :, r0 - 1:r0 + CH_ROWS - 1, :])
            nc.vector.tensor_copy(xq[:], xs[:])  # f32 -> bf16 (pads stay 0)

            for co_half in range(CO // 128):
                ps = psum_pool.tile([128, NFREE], F32, name="ps")
                for k9 in range(KH * KW):
                    kh, kw = divmod(k9, KW)
                    rhs = xq[:, kh: kh + ROWS_PER_TILE, kw: kw + W]
                    nc.tensor.matmul(
                        ps[:],
                        w_sb[:, k9, co_half * 128: (co_half + 1) * 128],
                        rhs,
                        start=(k9 == 0),
                        stop=(k9 == KH * KW - 1),
                    )
                ob = out_pool.tile([128, NFREE], F32, name="ob")
                if cfg["copy_engine"] == "scalar":
                    nc.scalar.copy(ob[:], ps[:])
                else:
                    nc.vector.tensor_copy(ob[:], ps[:])
                nc.sync.dma_start(
                    y_d[n, co_half * 128: (co_half + 1) * 128,
                        r0: r0 + ROWS_PER_TILE, :],
                    ob[:],
                )


def _emit_body(nc, pools, x_d, y_d, w_sb, cfg):
    """One full pass: conv of the core's BPC images."""
    xstage_pool, xpad_pool, out_pool, psum_pool = pools
    copy_i = 0
    for n in range(BPC):
        xs = xstage_pool.tile([CI, HP, WP], F32, name="xs")
        # zero the pad border (only the border: interior is DMA'd over)
        nc.vector.memset(xs[:, 0, :], 0.0)
        nc.vector.memset(xs[:, HP - 1, :], 0.0)
        nc.vector.memset(xs[:, 1:HP - 1, 0], 0.0)
        nc.vector.memset(xs[:, 1:HP - 1, WP - 1], 0.0)
        nc.sync.dma_start(xs[:, 1:H + 1, 1:W + 1], x_d[n])
        xq = xpad_pool.tile([CI, HP, WP], BF16, name="xq")
        nc.vector.tensor_copy(xq[:], xs[:])  # f32 -> bf16 cast

        for co_half in range(CO // 128):
            for t in range(N_ROW_TILES):
                ps = psum_pool.tile([128, NFREE], F32, name="ps")
                r0 = t * ROWS_PER_TILE
                for k9 in range(KH * KW):
                    kh, kw = divmod(k9, KW)
                    rhs = xq[:, r0 + kh: r0 + kh + ROWS_PER_TILE, kw: kw + W]
                    nc.tensor.matmul(
                        ps[:],
                        w_sb[:, k9, co_half * 128: (co_half + 1) * 128],
                        rhs,
                        start=(k9 == 0),
                        stop=(k9 == KH * KW - 1),
                    )
                ob = out_pool.tile([128, NFREE], F32, name="ob")
                ce = cfg["copy_engine"]
                if ce == "alternate":
                    ce = "vector" if copy_i % 2 == 0 else "scalar"
                copy_i += 1
                if ce == "vector":
                    nc.vector.tensor_copy(ob[:], ps[:])
                else:
                    nc.scalar.copy(ob[:], ps[:])
                nc.sync.dma_start(
                    y_d[n, co_half * 128: (co_half + 1) * 128,
                        r0: r0 + ROWS_PER_TILE, :],
                    ob[:],
                )


def build_program(static_reps: int = 1, **overrides) -> bass.Bass:
    cfg = dict(DEFAULT_CFG, **overrides)
    nc = bacc.Bacc(name="binconv2d")
    x_d = nc.dram_tensor("x", (BPC, CI, H, W), F32, kind="ExternalInput")
    w_d = nc.dram_tensor("w", (KH * KW, CI, CO), BF16, kind="ExternalInput")
    y_d = nc.dram_tensor("y", (BPC, CO, H, W), F32, kind="ExternalOutput")

    with tile.TileContext(nc) as tc:
        with (
            tc.tile_pool(name="wpool", bufs=1) as wpool,
            tc.tile_pool(name="xstage", bufs=cfg["xstage_bufs"]) as xstage_pool,
            tc.tile_pool(name="xpad", bufs=cfg["xpad_bufs"]) as xpad_pool,
            tc.tile_pool(name="outb", bufs=cfg["out_bufs"]) as out_pool,
            tc.tile_pool(name="psum", bufs=cfg["psum_bufs"], space="PSUM") as psum_pool,
        ):
            w_sb = wpool.tile([CI, KH * KW, CO], BF16)
            nc.sync.dma_start(w_sb[:], w_d[:].rearrange("k p c -> p k c"))
            pools = (xstage_pool, xpad_pool, out_pool, psum_pool)

            if cfg["chunked"]:
                chunk_tiles, chunk_pool = _alloc_chunk_tiles(tc, nc)
                state = {"first": 0, "mid": 0, "last": 0}
                for _ in range(static_reps):
                    _emit_body_chunked(nc, pools, x_d, y_d, w_sb, cfg,
                                       chunk_tiles, state)
                chunk_pool.release()
            else:
                for _ in range(static_reps):
                    _emit_body(nc, pools, x_d, y_d, w_sb, cfg)

    nc.finalize()
    return nc


def prep_weights(weights: np.ndarray) -> np.ndarray:
    """sign(weights) as bf16, laid out [kh*kw, ci, co]."""
    bw = np.sign(np.asarray(weights, dtype=np.float32))
    # (co, ci, kh, kw) -> (kh, kw, ci, co) -> (9, ci, co)
    bw = np.ascontiguousarray(bw.transpose(2, 3, 1, 0)).reshape(KH * KW, CI, CO)
    return bw.astype(ml_dtypes.bfloat16)


def make_in_maps(x: np.ndarray, weights: np.ndarray) -> list[dict]:
    x = np.ascontiguousarray(np.asarray(x, dtype=np.float32))
    w_l = prep_weights(weights)
    return [
        {"x": x[i * BPC:(i + 1) * BPC], "w": w_l}
        for i in range(N_CORES)
    ]


def kernel(x, weights) -> np.ndarray:
    nc = build_program()
    in_maps = make_in_maps(x, weights)
    res = run_bass_kernel_spmd(nc, in_maps, core_ids=list(range(N_CORES)))
    return np.concatenate([r["y"] for r in res.results], axis=0)


# revision 14
# speedup vs baseline: 1.1443x; 1.1443x over previous
"""Binary-weight 3x3 conv2d (stride 1, pad 1) on 8 TRN2 NeuronCores.

Reference computes y = conv2d(x, sign(weights)) in NCHW/OIHW, f32.
  x: (32, 128, 56, 56) f32, weights: (256, 128, 3, 3) f32 -> y: (32, 256, 56, 56) f32

Strategy (sharding hint: data-parallel on batch, weights replicated):
  - 8 cores x 4 images each.
  - Host side: binarize weights (sign -> exactly representable +-1 in bf16)
    and lay out as [kh*kw, ci, co] bf16.
  - Per core: each image is zero-padded to 58x58 in SBUF (bf16); the 3x3 conv
    becomes 9 shifted matmuls (lhsT = w[k9] in [ci=128, co=128] chunks,
    rhs = padded x slice [ci=128, 8 rows x 56 cols]) accumulating into PSUM.
  - PSUM f32 tile [128, 448] -> SBUF -> DMA to y.

x is cast to bf16 on-chip (weights are exact +-1, so the only quantization is
on x: ~1.7e-3 relative error on the output; accumulation is f32 in PSUM).
"""

import numpy as np
import ml_dtypes

import concourse.bass as bass
import concourse.bacc as bacc
import concourse.mybir as mybir
import concourse.tile as tile
from concourse.bass_utils import run_bass_kernel_spmd

N_CORES = 8
B, CI, H, W = 32, 128, 56, 56
CO = 256
KH = KW = 3
BPC = B // N_CORES          # images per core
HP, WP = H + 2, W + 2       # padded spatial
ROWS_PER_TILE = 8
N_ROW_TILES = H // ROWS_PER_TILE   # 7
NFREE = ROWS_PER_TILE * W          # 448 <= 512 (one PSUM bank)

F32 = mybir.dt.float32
BF16 = mybir.dt.bfloat16


DEFAULT_CFG = dict(
    psum_bufs=6, out_bufs=6, xstage_bufs=3, xpad_bufs=3,
    copy_engine="vector",        # "vector" | "scalar" | "alternate"
    chunked=True,                # per-row-tile x chunks (finer pipeline)
)

CH_ROWS = ROWS_PER_TILE + 2     # 10 padded rows per chunk


def _alloc_chunk_tiles(tc, nc):
    """Persistent x chunk tiles with one-time zeroed pad borders.

    A chunk holds the 10 padded input rows needed by one 8-row output tile.
    kind 'first' has the top pad row, 'last' the bottom pad row. Pad columns
    (0 and 57) are zeroed once; DMAs only ever write interior cells, so the
    zeros persist across reuse.
    """
    tiles = {"first": [], "mid": [], "last": []}
    pool = tc.alloc_tile_pool(name="chunks", bufs=1)
    seq = [0]

    def mk(kind):
        i = seq[0]
        seq[0] += 1
        xs = pool.tile([CI, CH_ROWS, WP], F32, name=f"xs_{kind}{i}")
        xq = pool.tile([CI, CH_ROWS, WP], BF16, name=f"xq_{kind}{i}")
        nc.vector.memset(xs[:, :, 0], 0.0)
        nc.vector.memset(xs[:, :, WP - 1], 0.0)
        if kind == "first":
            nc.vector.memset(xs[:, 0, :], 0.0)
        if kind == "last":
            nc.vector.memset(xs[:, CH_ROWS - 1, :], 0.0)
        return (xs, xq)

    for _ in range(2):
        tiles["first"].append(mk("first"))
    for _ in range(2):
        tiles["last"].append(mk("last"))
    for _ in range(4):
        tiles["mid"].append(mk("mid"))
    return tiles, pool


def _emit_body_chunked(nc, pools, x_d, y_d, w_sb, cfg, chunk_tiles, state):
    _, _, out_pool, psum_pool = pools
    for n in range(BPC):
        for t in range(N_ROW_TILES):
            r0 = t * ROWS_PER_TILE
            kind = "first" if t == 0 else ("last" if t == N_ROW_TILES - 1 else "mid")
            group = chunk_tiles[kind]
            xs, xq = group[state[kind] % len(group)]
            state[kind] += 1
            if kind == "first":
                nc.sync.dma_start(xs[:, 1:CH_ROWS, 1:W + 1],
                                  x_d[n, :, 0:CH_ROWS - 1, :])
            elif kind == "last":
                nc.sync.dma_start(xs[:, 0:CH_ROWS - 1, 1:W + 1],
                                  x_d[n, :, r0 - 1:H, :])
            else:
                nc.sync.dma_start(xs[:, :, 1:W + 1],
                                  x_d[n, :, r0 - 1:r0 + CH_ROWS - 1, :])
            nc.vector.tensor_copy(xq[:], xs[:])  # f32 -> bf16 (pads stay 0)

            for co_half in range(CO // 128):
                ps = psum_pool.tile([128, NFREE], F32, name="ps")
                for k9 in range(KH * KW):
                    kh, kw = divmod(k9, KW)
                    rhs = xq[:, kh: kh + ROWS_PER_TILE, kw: kw + W]
                    nc.tensor.matmul(
                        ps[:],
                        w_sb[:, k9, co_half * 128: (co_half + 1) * 128],
                        rhs,
                        start=(k9 == 0),
                        stop=(k9 == KH * KW - 1),
                    )
                ob = out_pool.tile([128, NFREE], F32, name="ob")
                if cfg["copy_engine"] == "scalar":
                    nc.scalar.copy(ob[:], ps[:])
                else:
                    nc.vector.tensor_copy(ob[:], ps[:])
                nc.sync.dma_start(
                    y_d[n, co_half * 128: (co_half + 1) * 128,
                        r0: r0 + ROWS_PER_TILE, :],
                    ob[:],
                )


def _emit_body(nc, pools, x_d, y_d, w_sb, cfg):
    """One full pass: conv of the core's BPC images."""
    xstage_pool, xpad_pool, out_pool, psum_pool = pools
    copy_i = 0
    for n in range(BPC):
        xs = xstage_pool.tile([CI, HP, WP], F32, name="xs")
        # zero the pad border (only the border: interior is DMA'd over)
        nc.vector.memset(xs[:, 0, :], 0.0)
        nc.vector.memset(xs[:, HP - 1, :], 0.0)
        nc.vector.memset(xs[:, 1:HP - 1, 0], 0.0)
        nc.vector.memset(xs[:, 1:HP - 1, WP - 1], 0.0)
        nc.sync.dma_start(xs[:, 1:H + 1, 1:W + 1], x_d[n])
        xq = xpad_pool.tile([CI, HP, WP], BF16, name="xq")
        nc.vector.tensor_copy(xq[:], xs[:])  # f32 -> bf16 cast

        for co_half in range(CO // 128):
            for t in range(N_ROW_TILES):
                ps = psum_pool.tile([128, NFREE], F32, name="ps")
                r0 = t * ROWS_PER_TILE
                for k9 in range(KH * KW):
                    kh, kw = divmod(k9, KW)
                    rhs = xq[:, r0 + kh: r0 + kh + ROWS_PER_TILE, kw: kw + W]
                    nc.tensor.matmul(
                        ps[:],
                        w_sb[:, k9, co_half * 128: (co_half + 1) * 128],
                        rhs,
                        start=(k9 == 0),
                        stop=(k9 == KH * KW - 1),
                    )
                ob = out_pool.tile([128, NFREE], F32, name="ob")
                ce = cfg["copy_engine"]
                if ce == "alternate":
                    ce = "vector" if copy_i % 2 == 0 else "scalar"
                copy_i += 1
                if ce == "vector":
                    nc.vector.tensor_copy(ob[:], ps[:])
                else:
                    nc.scalar.copy(ob[:], ps[:])
                nc.sync.dma_start(
                    y_d[n, co_half * 128: (co_half + 1) * 128,
                        r0: r0 + ROWS_PER_TILE, :],
                    ob[:],
                )


def build_program(static_reps: int = 1, **overrides) -> bass.Bass:
    cfg = dict(DEFAULT_CFG, **overrides)
    nc = bacc.Bacc(name="binconv2d")
    x_d = nc.dram_tensor("x", (BPC, CI, H, W), F32, kind="ExternalInput")
    w_d = nc.dram_tensor("w", (KH * KW, CI, CO), BF16, kind="ExternalInput")
    y_d = nc.dram_tensor("y", (BPC, CO, H, W), F32, kind="ExternalOutput")

    with tile.TileContext(nc) as tc:
        with (
            tc.tile_pool(name="wpool", bufs=1) as wpool,
            tc.tile_pool(name="xstage", bufs=cfg["xstage_bufs"]) as xstage_pool,
            tc.tile_pool(name="xpad", bufs=cfg["xpad_bufs"]) as xpad_pool,
            tc.tile_pool(name="outb", bufs=cfg["out_bufs"]) as out_pool,
            tc.tile_pool(name="psum", bufs=cfg["psum_bufs"], space="PSUM") as psum_pool,
        ):
            w_sb = wpool.tile([CI, KH * KW, CO], BF16)
            nc.sync.dma_start(w_sb[:], w_d[:].rearrange("k p c -> p k c"))
            pools = (xstage_pool, xpad_pool, out_pool, psum_pool)

            if cfg["chunked"]:
                chunk_tiles, chunk_pool = _alloc_chunk_tiles(tc, nc)
                state = {"first": 0, "mid": 0, "last": 0}
                for _ in range(static_reps):
                    _emit_body_chunked(nc, pools, x_d, y_d, w_sb, cfg,
                                       chunk_tiles, state)
                chunk_pool.release()
            else:
                for _ in range(static_reps):
                    _emit_body(nc, pools, x_d, y_d, w_sb, cfg)

    nc.finalize()
    return nc


def prep_weights(weights: np.ndarray) -> np.ndarray:
    """sign(weights) as bf16, laid out [kh*kw, ci, co]."""
    bw = np.sign(np.asarray(weights, dtype=np.float32))
    # (co, ci, kh, kw) -> (kh, kw, ci, co) -> (9, ci, co)
    bw = np.ascontiguousarray(bw.transpose(2, 3, 1, 0)).reshape(KH * KW, CI, CO)
    return bw.astype(ml_dtypes.bfloat16)


def make_in_maps(x: np.ndarray, weights: np.ndarray) -> list[dict]:
    x = np.ascontiguousarray(np.asarray(x, dtype=np.float32))
    w_l = prep_weights(weights)
    return [
        {"x": x[i * BPC:(i + 1) * BPC], "w": w_l}
        for i in range(N_CORES)
    ]


def kernel(x, weights) -> np.ndarray:
    nc = build_program()
    in_maps = make_in_maps(x, weights)
    res = run_bass_kernel_spmd(nc, in_maps, core_ids=list(range(N_CORES)))
    return np.concatenate([r["y"] for r in res.results], axis=0)
